# revision 4
# baseline (speedup 1.0000x reference)
"""Trainium2 Bass kernel for nn_GroupAttention (tree-transformer group attention).

Math (per batch b):
  z   = (c - mu)/(std_ddof1 + 1e-6)            (LayerNorm; gamma folded into M)
  s   = z @ Mt @ z.T                           (Mt = Wq'^T Wk' / 512, host-precomputed;
                                                row-constant bias terms cancel in softmax,
                                                column term v is zero for zero biases --
                                                asserted on host, else folded into madj)
  E   = exp(s + madj)        madj in {0,-60};  si = 1/rowsum(E)
  nb  = prior + sqrt(E*E^T*si2_i*si2_j + eps2) (si2 = si*(1-prior), eps2=(1-prior)^2*1e-9)
  L_i = log(nb[i,i+1] + 1e-9);  P = exclusive prefix sum of L
  g[i,j] = exp(-|P[j]-P[i]|) (i != j),  g[i,i] = nb[i,i]

The [S,S] tri-matmul chain in the reference collapses exactly to the prefix-sum
form above. Sharding: data-parallel over batch, 1 batch element per core (B=8).
Both transposes (z and E) run on the DMA XBAR (SBUF->SBUF, bf16), keeping the
PE free for the two 1024^3 matmul chains.
"""
import sys

sys.path.insert(0, "/opt/trn_rl_repo")

import numpy as np
import ml_dtypes

from concourse import bass, bacc, mybir, tile, masks
from concourse.bass_utils import run_bass_kernel_spmd

B, S, D = 8, 1024, 1024
P = 128
NT = S // P  # 8 row tiles
F32 = mybir.dt.float32
BF16 = mybir.dt.bfloat16
AF = mybir.ActivationFunctionType
OP = mybir.AluOpType
N_CORES = 8


def build_bass(prior: float):
    nc = bacc.Bacc(
        "TRN2",
        target_bir_lowering=False,
        debug=False,
        enable_asserts=False,
        num_devices=N_CORES,
    )

    omp = 1.0 - prior
    eps2 = float(omp * omp * 1e-9)

    ctx_d = nc.dram_tensor("ctx", [S, D], BF16, kind="ExternalInput").ap()
    madj_d = nc.dram_tensor("madj", [S, S], BF16, kind="ExternalInput").ap()
    m_d = nc.dram_tensor("mt", [D, D], BF16, kind="ExternalInput").ap()
    nout_d = nc.dram_tensor("n_out", [S, S], BF16, kind="ExternalOutput").ap()
    gout_d = nc.dram_tensor("g_out", [S, S], BF16, kind="ExternalOutput").ap()

    ctx_r = ctx_d.rearrange("(t p) d -> p t d", p=P)
    madj_r = madj_d.rearrange("(t p) s -> p t s", p=P)
    m_r = m_d.rearrange("(c p) e -> p c e", p=P)
    nout_r = nout_d.rearrange("(t p) s -> p t s", p=P)
    gout_r = gout_d.rearrange("(t p) s -> p t s", p=P)

    with tile.TileContext(nc) as tc:
        with (
            tc.tile_pool(name="consts", bufs=1) as cpool,
            tc.tile_pool(name="main", bufs=1) as mpool,
            tc.tile_pool(name="scratch", bufs=3) as spool,
            tc.tile_pool(name="gout", bufs=2) as gpool,
            tc.tile_pool(name="psum", bufs=2, space="PSUM") as ppool,
        ):
            # ---- constants ----
            id_f32 = cpool.tile([P, P], F32, tag="id_f32")
            masks.make_identity(nc, id_f32[:])
            strict8 = cpool.tile([NT, NT], F32, tag="strict8")
            nc.gpsimd.memset(strict8[:], 1.0)
            nc.gpsimd.affine_select(
                out=strict8[:], in_=strict8[:], compare_op=OP.is_gt,
                fill=0.0, base=0, pattern=[[1, NT]], channel_multiplier=-1,
            )
            zeros8 = cpool.tile([NT, P], F32, tag="zeros8")
            nc.vector.memset(zeros8[:], 0.0)
            eps2t = cpool.tile([P, 1], F32, tag="eps2t")
            nc.vector.memset(eps2t[:], eps2)
            prl = cpool.tile([P, 1], F32, tag="prl")
            nc.vector.memset(prl[:], float(prior + 1e-9))
            # sel[k, t, m] = 1 iff k == t : row-selector weights
            sel = cpool.tile([NT, NT, P], F32, tag="sel")
            nc.gpsimd.memset(sel[:], 1.0)
            nc.gpsimd.affine_select(
                out=sel[:], in_=sel[:], compare_op=OP.is_equal,
                fill=0.0, base=0, pattern=[[1, NT], [0, P]], channel_multiplier=-1,
            )

            # ---- small whole-kernel tiles ----
            ssum = mpool.tile([P, NT], F32, tag="ssum")
            ssq = mpool.tile([P, NT], F32, tag="ssq")
            mu = mpool.tile([P, NT], F32, tag="mu")
            istd = mpool.tile([P, NT], F32, tag="istd")
            tmp8 = mpool.tile([P, NT], F32, tag="tmp8")
            rs2 = mpool.tile([P, 2 * NT], F32, tag="rs2")
            rs = mpool.tile([P, NT], F32, tag="rs")
            si2 = mpool.tile([P, NT], F32, tag="si2")
            lmat = mpool.tile([P, NT], F32, tag="lmat")
            pcol = mpool.tile([P, NT], F32, tag="pcol")
            lrows = mpool.tile([NT, P], F32, tag="lrows")
            pincl = mpool.tile([NT, P], F32, tag="pincl")
            pex = mpool.tile([NT, P], F32, tag="pex")
            offs = mpool.tile([NT, 1], F32, tag="offs")
            sirow = mpool.tile([NT, P], F32, tag="sirow")
            pb = mpool.tile([P, S], F32, tag="pb")
            sjb = mpool.tile([P, S], F32, tag="sjb")
            e_sb = mpool.tile([P, NT, S], BF16, tag="e")
            et_sb = mpool.tile([P, NT, S], BF16, tag="et")
            nb_sb = mpool.tile([P, NT, S], BF16, tag="nb")

            with tc.tile_pool(name="stage2", bufs=1) as s2pool:
                zt_sb = s2pool.tile([P, NT, S], BF16, tag="zt")
                tt_sb = s2pool.tile([P, NT, S], BF16, tag="tt")
                madj_sb = s2pool.tile([P, NT, S], BF16, tag="madj")

                with tc.tile_pool(name="stage1", bufs=1) as s1pool:
                    ctx_sb = s1pool.tile([P, NT, D], BF16, tag="ctx")
                    m_sb = s1pool.tile([P, NT, D], BF16, tag="m")

                    # ---- loads ----
                    for t in range(NT):
                        nc.sync.dma_start(out=ctx_sb[:, t], in_=ctx_r[:, t])
                    for c in range(NT):
                        nc.sync.dma_start(out=m_sb[:, c], in_=m_r[:, c])
                    for t in range(NT):
                        nc.sync.dma_start(out=madj_sb[:, t], in_=madj_r[:, t])

                    # ---- layernorm stats ----
                    for t in range(NT):
                        nc.vector.tensor_reduce(
                            out=ssum[:, t : t + 1], in_=ctx_sb[:, t],
                            axis=mybir.AxisListType.X, op=OP.add,
                        )
                        scr = spool.tile([P, D], BF16, tag="scr_bf")
                        nc.vector.scalar_tensor_tensor(
                            out=scr[:], in0=ctx_sb[:, t], scalar=1.0,
                            in1=ctx_sb[:, t], op0=OP.mult, op1=OP.mult,
                            accum_out=ssq[:, t : t + 1],
                        )
                    # mu = ssum/D ; var = ssq/(D-1) - (D/(D-1)) mu^2
                    nc.vector.tensor_scalar(
                        out=mu[:], in0=ssum[:], scalar1=1.0 / D, scalar2=None,
                        op0=OP.mult,
                    )
                    nc.vector.tensor_mul(out=tmp8[:], in0=mu[:], in1=mu[:])
                    nc.vector.tensor_scalar(
                        out=ssq[:], in0=ssq[:], scalar1=1.0 / (D - 1), scalar2=None,
                        op0=OP.mult,
                    )
                    nc.vector.scalar_tensor_tensor(
                        out=tmp8[:], in0=tmp8[:], scalar=-float(D) / (D - 1),
                        in1=ssq[:], op0=OP.mult, op1=OP.add,
                    )
                    nc.scalar.activation(out=tmp8[:], in_=tmp8[:], func=AF.Sqrt)
                    nc.vector.tensor_scalar(
                        out=tmp8[:], in0=tmp8[:], scalar1=1e-6, scalar2=None,
                        op0=OP.add,
                    )
                    nc.vector.reciprocal(out=istd[:], in_=tmp8[:])

                    # ---- normalize in place, then DMA-XBAR transpose -> zt ----
                    for t in range(NT):
                        nc.vector.tensor_scalar(
                            out=ctx_sb[:, t], in0=ctx_sb[:, t],
                            scalar1=mu[:, t : t + 1], scalar2=istd[:, t : t + 1],
                            op0=OP.subtract, op1=OP.mult,
                        )
                        nc.sync.dma_start(
                            out=zt_sb[:, :, t * P : (t + 1) * P],
                            in_=ctx_sb[:, t], transpose=True,
                        )

                    # ---- tt = Mt^T @ zt  (projection chain) ----
                    for m in range(NT):
                        for h in range(2):
                            pq = ppool.tile([P, 512], F32, tag="mm")
                            for k in range(NT):
                                nc.tensor.matmul(
                                    out=pq[:],
                                    lhsT=m_sb[:, k, m * P : (m + 1) * P],
                                    rhs=zt_sb[:, k, h * 512 : (h + 1) * 512],
                                    start=(k == 0), stop=(k == NT - 1),
                                )
                            nc.scalar.copy(
                                out=tt_sb[:, m, h * 512 : (h + 1) * 512], in_=pq[:],
                            )

                # ---- scores + masked exp (E), transpose E via DMA XBAR ----
                for qt in range(NT):
                    for h in range(2):
                        ps = ppool.tile([P, 512], F32, tag="mm")
                        for m in range(NT):
                            nc.tensor.matmul(
                                out=ps[:],
                                lhsT=tt_sb[:, m, qt * P : (qt + 1) * P],
                                rhs=zt_sb[:, m, h * 512 : (h + 1) * 512],
                                start=(m == 0), stop=(m == NT - 1),
                            )
                        w = spool.tile([P, 512], F32, tag="wsc")
                        nc.vector.scalar_tensor_tensor(
                            out=w[:], in0=ps[:], scalar=1.0,
                            in1=madj_sb[:, qt, h * 512 : (h + 1) * 512],
                            op0=OP.mult, op1=OP.add,
                        )
                        nc.scalar.activation(
                            out=e_sb[:, qt, h * 512 : (h + 1) * 512], in_=w[:],
                            func=AF.Exp,
                            accum_out=rs2[:, qt * 2 + h : qt * 2 + h + 1],
                        )
                    nc.sync.dma_start(
                        out=et_sb[:, :, qt * P : (qt + 1) * P],
                        in_=e_sb[:, qt], transpose=True,
                    )

            # stage1/stage2 pools closed
            rs2v = rs2[:].rearrange("p (t two) -> p t two", two=2)
            nc.vector.tensor_add(out=rs[:], in0=rs2v[:, :, 0], in1=rs2v[:, :, 1])
            nc.vector.reciprocal(out=rs[:], in_=rs[:])
            nc.vector.tensor_scalar(
                out=si2[:], in0=rs[:], scalar1=omp, scalar2=None, op0=OP.mult,
            )

            # ---- sjb[p, j] = si2[j]  (broadcast via row-selector matmul) ----
            pt = ppool.tile([P, 512], F32, tag="sm")
            nc.tensor.transpose(out=pt[0:NT, 0:P], in_=si2[:], identity=id_f32[:])
            nc.scalar.copy(out=sirow[:], in_=pt[0:NT, 0:P])
            for g4 in range(2):
                pt = ppool.tile([P, 512], F32, tag="sm")
                for j in range(4):
                    t = g4 * 4 + j
                    nc.tensor.matmul(
                        out=pt[:, j * P : (j + 1) * P], lhsT=sel[:, t, :],
                        rhs=sirow[:], start=True, stop=True,
                    )
                nc.scalar.copy(out=sjb[:, g4 * 512 : (g4 + 1) * 512], in_=pt[:])

            # ---- L_i = log(nb[i,i+1] + 1e-9) from cheap superdiag strips ----
            for t in range(NT):
                wdt = P if t < NT - 1 else P - 1
                base = t * P + 1
                ystr = spool.tile([P, P], F32, tag="ystr")
                nc.vector.scalar_tensor_tensor(
                    out=ystr[:, :wdt], in0=e_sb[:, t, base : base + wdt],
                    scalar=si2[:, t : t + 1],
                    in1=et_sb[:, t, base : base + wdt],
                    op0=OP.mult, op1=OP.mult,
                )
                nc.vector.tensor_mul(
                    out=ystr[:, :wdt], in0=ystr[:, :wdt],
                    in1=sjb[:, base : base + wdt],
                )
                nc.scalar.activation(
                    out=ystr[:, :wdt], in_=ystr[:, :wdt], func=AF.Sqrt, bias=eps2t[:],
                )
                dscr = spool.tile([P, P], F32, tag="dscr")
                nc.gpsimd.memset(dscr[:], 0.0)
                nc.gpsimd.affine_select(
                    out=dscr[:, :wdt], in_=ystr[:, :wdt],
                    compare_op=OP.is_equal, fill=0.0, base=0,
                    pattern=[[-1, wdt]], channel_multiplier=1,
                )
                nc.vector.tensor_reduce(
                    out=lmat[:, t : t + 1], in_=dscr[:],
                    axis=mybir.AxisListType.X, op=OP.add,
                )
            nc.scalar.activation(
                out=lmat[:], in_=lmat[:], func=AF.Ln, bias=prl[:],
            )

            # ---- prefix sums P (exclusive) in [NT, P] row layout ----
            pt = ppool.tile([P, 512], F32, tag="sm")
            nc.tensor.transpose(out=pt[0:NT, 0:P], in_=lmat[:], identity=id_f32[:])
            nc.scalar.copy(out=lrows[:], in_=pt[0:NT, 0:P])
            nc.vector.tensor_tensor_scan(
                out=pincl[:], data0=lrows[:], data1=zeros8[:],
                initial=0.0, op0=OP.add, op1=OP.add,
            )
            pt = ppool.tile([P, 512], F32, tag="sm")
            nc.tensor.matmul(
                out=pt[0:NT, 0:1], lhsT=strict8[:], rhs=pincl[:, P - 1 : P],
                start=True, stop=True,
            )
            nc.scalar.copy(out=offs[:], in_=pt[0:NT, 0:1])
            nc.vector.scalar_tensor_tensor(
                out=pex[:], in0=pincl[:], scalar=offs[:, 0:1],
                in1=lrows[:], op0=OP.add, op1=OP.subtract,
            )
            # pb[p, j] = P[j] (broadcast); pcol[p, t] = P[t*128+p]
            for g4 in range(2):
                pt = ppool.tile([P, 512], F32, tag="sm")
                for j in range(4):
                    t = g4 * 4 + j
                    nc.tensor.matmul(
                        out=pt[:, j * P : (j + 1) * P], lhsT=sel[:, t, :],
                        rhs=pex[:], start=True, stop=True,
                    )
                nc.scalar.copy(out=pb[:, g4 * 512 : (g4 + 1) * 512], in_=pt[:])
            pt = ppool.tile([P, 512], F32, tag="sm")
            nc.tensor.transpose(
                out=pt[0:P, 0:NT], in_=pex[:], identity=id_f32[0:NT, 0:NT]
            )
            nc.scalar.copy(out=pcol[:], in_=pt[0:P, 0:NT])

            # ---- nb tiles and g tiles, interleaved ----
            for t in range(NT):
                # nb = prior + sqrt(e*et*si2_i*si2_j + eps2)
                y = spool.tile([P, S], F32, tag="ynb")
                nc.vector.scalar_tensor_tensor(
                    out=y[:], in0=e_sb[:, t], scalar=si2[:, t : t + 1],
                    in1=et_sb[:, t], op0=OP.mult, op1=OP.mult,
                )
                nc.vector.tensor_mul(out=y[:], in0=y[:], in1=sjb[:])
                nc.scalar.activation(out=y[:], in_=y[:], func=AF.Sqrt, bias=eps2t[:])
                nc.scalar.activation(
                    out=nb_sb[:, t], in_=y[:], func=AF.Copy, bias=prior,
                )
                nc.sync.dma_start(out=nout_r[:, t], in_=nb_sb[:, t])

                # g = exp(-|P[j]-P[i]|) off-diag; diag <- nb
                gf = gpool.tile([P, S], F32, tag="gf")
                nc.vector.tensor_scalar(
                    out=gf[:], in0=pb[:], scalar1=pcol[:, t : t + 1],
                    scalar2=None, op0=OP.subtract,
                )
                nc.scalar.activation(out=gf[:], in_=gf[:], func=AF.Abs)
                # knock the diagonal of the input to +100 so exp(-x) ~ 0 there
                nc.gpsimd.affine_select(
                    out=gf[:, t * P : (t + 1) * P],
                    in_=gf[:, t * P : (t + 1) * P],
                    compare_op=OP.not_equal, fill=100.0, base=0,
                    pattern=[[-1, P]], channel_multiplier=1,
                )
                gt = gpool.tile([P, S], BF16, tag="gt")
                nc.scalar.activation(out=gt[:], in_=gf[:], func=AF.Exp, scale=-1.0)
                dnb = spool.tile([P, P], BF16, tag="dnb")
                nc.gpsimd.memset(dnb[:], 0.0)
                nc.gpsimd.affine_select(
                    out=dnb[:], in_=nb_sb[:, t, t * P : (t + 1) * P],
                    compare_op=OP.is_equal, fill=0.0, base=0,
                    pattern=[[-1, P]], channel_multiplier=1,
                )
                nc.vector.tensor_add(
                    out=gt[:, t * P : (t + 1) * P],
                    in0=gt[:, t * P : (t + 1) * P], in1=dnb[:],
                )
                nc.sync.dma_start(out=gout_r[:, t], in_=gt[:])

    return nc


def _prepare_inputs(inputs):
    context = np.asarray(inputs["context"], dtype=np.float32)
    adj = np.asarray(inputs["adj_mat"])
    prior = float(np.asarray(inputs["prior"]))
    Wk = np.asarray(inputs["Wk"], dtype=np.float32)
    Wq = np.asarray(inputs["Wq"], dtype=np.float32)
    bk = np.asarray(inputs["bk"], dtype=np.float32)
    bq = np.asarray(inputs["bq"], dtype=np.float32)
    gamma = np.asarray(inputs["ln_gamma"], dtype=np.float32)
    beta = np.asarray(inputs["ln_beta"], dtype=np.float32)

    ctx_bf = context.astype(ml_dtypes.bfloat16)
    # fold gamma into the weights; fold the 1/(d_model/2) score scale into Mt
    wqp = Wq * gamma[None, :]
    wkp = Wk * gamma[None, :]
    mt = np.ascontiguousarray((wqp.T @ wkp) / (D / 2)).astype(ml_dtypes.bfloat16)
    bqp = bq + beta @ Wq.T
    bkp = bk + beta @ Wk.T
    madj = (adj == 0).astype(np.float32) * (-60.0)
    if np.abs(bqp).max() > 0 or np.abs(bkp).max() > 0:
        # column bias term of the scores does not cancel in softmax; fold the
        # exact value into the additive mask (host layernorm, cheap vs matmuls)
        mu_h = context.mean(axis=-1, keepdims=True)
        sd_h = context.std(axis=-1, keepdims=True, ddof=1)
        z_h = (context - mu_h) / (sd_h + 1e-6)
        v = z_h @ (wkp.T @ bqp) / (D / 2)  # [B, S]
        madj = madj + v[:, None, :]
    madj = madj.astype(ml_dtypes.bfloat16)

    in_maps = []
    for b in range(N_CORES):
        in_maps.append(
            {
                "ctx": np.ascontiguousarray(ctx_bf[b]),
                "madj": np.ascontiguousarray(madj[b]),
                "mt": mt,
            }
        )
    return prior, in_maps


def _run(inputs, trace=False):
    prior, in_maps = _prepare_inputs(inputs)
    nc = build_bass(prior)
    if not nc.is_finalized():
        nc.finalize()
    res = run_bass_kernel_spmd(nc, in_maps, list(range(N_CORES)), trace=trace)
    g = np.stack(
        [np.asarray(res.results[b]["g_out"], dtype=np.float32) for b in range(N_CORES)]
    )
    n = np.stack(
        [np.asarray(res.results[b]["n_out"], dtype=np.float32) for b in range(N_CORES)]
    )
    return (g, n), res


def kernel(**inputs):
    out, _ = _run(inputs, trace=False)
    return out
